# revision 48
# baseline (speedup 1.0000x reference)
"""Causal multi-head attention (B=4, S=2048, HID=1024, 16 heads x 64) with RoPE
on 8 TRN2 NeuronCores.

Sharding: core c -> batch b = c//2, head-group hg = c%2 (8 heads each).

Per core: Q/K projections are emitted in four per-q-chunk stages interleaved
with the attention blocks, so projection matmuls fill the PE while the ACT
engine grinds through the exp()s of the previous attention block. Scores are
computed transposed sT[kk, q] with both heads of a pair row-packed into one PE
pass, exp on ACT trimmed to the causal column range, causal masking via
affine_select on the 128-wide diagonal triangle only, V carries a ones column
so the softmax denominator lands in ctx psum row 64, denominator broadcast via
K=1 ones-matmuls on the PE (no DRAM bounce), o_proj and a pair ReduceScatter
(fp16) pipelined per 512-query chunk with the last chunk's RS split in half to
shorten the tail.

All matmuls run in fp16 (fp32 PSUM accumulation). Constants live in per-chunk
tiles so consumers only wait for the DMAs they actually need.
"""
import os as _os
import numpy as np
from contextlib import ExitStack

import concourse.bass as bass
import concourse.tile as tile
import concourse.mybir as mybir
from concourse import bacc
from concourse.alu_op_type import AluOpType
from concourse.bass_utils import run_bass_kernel_spmd
import concourse.bass_utils as _bu

LDW_OPT = _os.environ.get("KLDW", "0") == "1"
if LDW_OPT and not getattr(_bu, "_ldw_patched", False):
    _orig_run_command = _bu.run_command
    def _run_command_ldwopt(argv, **kw):
        argv = ["--enable-ldw-opt=true" if a == "--enable-ldw-opt=false" else a
                for a in argv]
        return _orig_run_command(argv, **kw)
    _bu.run_command = _run_command_ldwopt
    _bu._ldw_patched = True

F32 = mybir.dt.float32
F16 = mybir.dt.float16
BF16 = mybir.dt.bfloat16
F8 = mybir.dt.float8e4
MM_DT = BF16 if _os.environ.get("KMM", "f16") == "bf16" else F16
DR = mybir.MatmulPerfMode.DoubleRow
AF = mybir.ActivationFunctionType
Alu = AluOpType
W8S = 32.0       # fp8 Q/K weight pre-scale (keeps sigma~0.02 out of subnormals)

B, S, HID = 4, 2048, 1024
NH, HD = 16, 64
SCALE = 1.0 / np.sqrt(HD)
ROPE_BASE = 10000.0
NCORES = 8
HPC = 8          # heads per core
JC = 512         # head dims per core
NJ = 4           # q chunks of 512
NT = 16          # kk tiles of 128
NSC = 4          # s chunks of 512 for projections
NHC = 8          # hid chunks of 128 (contraction)

_PROGRAM = None


def build():
    nc = bacc.Bacc("TRN2", target_bir_lowering=False, debug=False)

    hsT_d = nc.declare_dram_parameter("hsT", [HID, S], MM_DT, isOutput=False)
    hsT8_d = nc.declare_dram_parameter("hsT8", [HID, S], F8, isOutput=False)
    wq8_d = nc.declare_dram_parameter("wq8T", [HID, JC], F8, isOutput=False)
    wk8_d = nc.declare_dram_parameter("wk8T", [HID, JC], F8, isOutput=False)
    wv_d = nc.declare_dram_parameter("wvT", [HID, JC], MM_DT, isOutput=False)
    wo_d = nc.declare_dram_parameter("woT", [JC, HID], MM_DT, isOutput=False)
    cos_d = nc.declare_dram_parameter("cosT2", [128, S], MM_DT, isOutput=False)
    sin_d = nc.declare_dram_parameter("sinT2", [128, S], MM_DT, isOutput=False)
    trimask_d = nc.declare_dram_parameter("trimask", [128, 128], MM_DT, isOutput=False)
    out_d = nc.declare_dram_parameter("out", [S // 2, HID], MM_DT, isOutput=True)

    cc_in = nc.dram_tensor("cc_in", [S, HID], MM_DT)
    # last q chunk's ReduceScatter is split in four to shorten the tail
    cc_out = [nc.dram_tensor(f"cc_out{j}", [S // 8, HID], MM_DT) for j in range(NJ - 1)]
    cc_out3 = [nc.dram_tensor(f"cc_out3{h}", [S // 32, HID], MM_DT) for h in range(4)]
    # tiny scratch collective fired at t~0: absorbs the CC-path init cost
    # (observed 15-100us on the first real collective) during the projections
    cc_warm_in = nc.dram_tensor("cc_warm_in", [2, 128], MM_DT)
    cc_warm_out = nc.dram_tensor("cc_warm_out", [1, 128], MM_DT)

    with ExitStack() as ctx:
        tc = ctx.enter_context(tile.TileContext(nc, num_cores=NCORES))
        consts = ctx.enter_context(tc.tile_pool(name="consts", bufs=1))
        rt = ctx.enter_context(tc.tile_pool(name="rt", bufs=6))
        ptp = ctx.enter_context(tc.tile_pool(name="ptp", bufs=6))
        misc = ctx.enter_context(tc.tile_pool(name="misc", bufs=2))
        outp = ctx.enter_context(tc.tile_pool(name="outp", bufs=5))
        psum = ctx.enter_context(tc.tile_pool(name="psum", bufs=2, space="PSUM"))

        # ---- constants in per-chunk tiles (fine-grained DMA deps) ----
        # Q/K projections run in fp8-DR (pair layout [128, 2, .]); V needs
        # fp16 accuracy so hsT is shipped in both precisions.
        hsT8_c = [consts.tile([128, 2, S], F8, tag=f"hsT8{c}", name=f"hsT8{c}")
                  for c in range(NHC // 2)]
        wq8_c = [consts.tile([128, 2, JC], F8, tag=f"wq8{c}", name=f"wq8{c}")
                 for c in range(NHC // 2)]
        wk8_c = [consts.tile([128, 2, JC], F8, tag=f"wk8{c}", name=f"wk8{c}")
                 for c in range(NHC // 2)]
        hsT_c = [consts.tile([128, S], MM_DT, tag=f"hsT{hc}", name=f"hsT{hc}")
                 for hc in range(NHC)]
        wv_c = [consts.tile([128, JC], MM_DT, tag=f"wv{hc}", name=f"wv{hc}")
                for hc in range(NHC)]
        # round-robin the issue queues so the boot DMA stream isn't paced by
        # a single engine's ~0.6us-per-descriptor issue rate
        _qs = [nc.sync, nc.gpsimd, nc.scalar]
        _qi = [0]
        def _dma(out, in_):
            _qs[_qi[0] % 3].dma_start(out=out, in_=in_)
            _qi[0] += 1
        for c in range(NHC // 2):
            psl = slice(c * 256, (c + 1) * 256)
            _dma(wq8_c[c][:], wq8_d[psl, :].rearrange("(o p) j -> p o j", p=128))
            _dma(hsT8_c[c][:], hsT8_d[psl, :].rearrange("(o p) j -> p o j", p=128))
            _dma(wk8_c[c][:], wk8_d[psl, :].rearrange("(o p) j -> p o j", p=128))
        for hc in range(NHC):
            csl = slice(hc * 128, (hc + 1) * 128)
            _dma(hsT_c[hc][:], hsT_d[csl, :])
            _dma(wv_c[hc][:], wv_d[csl, :])
        cos2 = consts.tile([128, S], MM_DT, tag="cos2")
        sin2 = consts.tile([128, S], MM_DT, tag="sin2")
        _dma(cos2[:], cos_d[:])
        _dma(sin2[:], sin_d[:])
        # upper triangle (keep q >= kk) for DVE-side masking of block 0,
        # whose gpsimd affine_selects would otherwise queue behind the
        # collective warmup triggers on the gpsimd FIFO
        trimask = consts.tile([128, 128], MM_DT, tag="trimask")
        _dma(trimask[:], trimask_d[:])
        wo = consts.tile([128, 4, HID], MM_DT, tag="wo")
        _dma(wo[:], wo_d[:].rearrange("(c p) j -> p c j", p=128))
        # all-ones stationary tile: K=1 matmuls replicate a denominator row
        # across 64 output partitions
        ones1 = consts.tile([128, 128], MM_DT, tag="ones1")
        nc.vector.memset(ones1[:], 1.0)

        # rope outputs in per-q-chunk tiles so a later projection stage's
        # writes never false-conflict with an earlier attention block's reads
        qrope = [[consts.tile([128, 512], MM_DT, tag=f"qr{i}_{sc}",
                              name=f"qrope{i}_{sc}") for sc in range(NSC)]
                 for i in range(4)]
        krope = [[consts.tile([128, 512], MM_DT, tag=f"kr{i}_{sc}",
                              name=f"krope{i}_{sc}") for sc in range(NSC)]
                 for i in range(4)]
        # V in per-st-group tiles (group g holds kk tiles 4g..4g+3)
        v_sb = [consts.tile([128, 4, HPC, HD + 1], MM_DT, tag=f"v_sb{g}",
                            name=f"v_sb{g}") for g in range(4)]
        for g in range(4):
            nc.vector.memset(v_sb[g][:, :, :, HD:HD + 1], 1.0)

        ctx_sb = [consts.tile([128, S], MM_DT, tag=f"ctx{i}", name=f"ctx_sb{i}")
                  for i in range(4)]

        def proj_quantum(sc, hp, which, act_copy=False):
            """One Q or K projection group + RoPE for (q-chunk sc, head pair hp)."""
            ssl = slice(sc * 512, (sc + 1) * 512)
            jcol = hp * 128
            wt = wq8_c if which == "q" else wk8_c
            dest = qrope if which == "q" else krope

            def emit():
                ps_raw = psum.tile([128, 512], F32, tag="mm")
                for c in range(NHC // 2):
                    nc.tensor.matmul(
                        out=ps_raw[:],
                        lhsT=wt[c][:, :, jcol:jcol + 128],
                        rhs=hsT8_c[c][:, :, ssl],
                        start=(c == 0), stop=(c == NHC // 2 - 1),
                        perf_mode=DR,
                    )
                raw_sb = misc.tile([128, 512], MM_DT, tag="qraw", bufs=4,
                                   name=f"raw_{which}{hp}_{sc}")
                # preamble drains via ACT (idle there); pumped stages via DVE
                # so the psum rotation never waits behind the exp grind
                if act_copy:
                    nc.scalar.copy(out=raw_sb[:], in_=ps_raw[:])
                else:
                    nc.vector.tensor_copy(out=raw_sb[:], in_=ps_raw[:])
                rot_sb = misc.tile([128, 512], MM_DT, tag="qrot", bufs=4,
                                   name=f"rot_{which}{hp}_{sc}")
                # alternate the trigger queue so neither engine bottlenecks;
                # the preamble avoids gpsimd (blocked by collective warmups)
                alt_q = nc.scalar if act_copy else nc.gpsimd
                for hl in range(2):
                    b0 = 64 * hl
                    # rot rows 0:32 <- raw rows 1,3,..,63 (odd)
                    nc.sync.dma_start(
                        out=rot_sb[b0:b0 + 32, :],
                        in_=raw_sb[b0 + 1:b0 + 64:2, :],
                    )
                    alt_q.dma_start(
                        out=rot_sb[b0 + 32:b0 + 64, :],
                        in_=raw_sb[b0:b0 + 63:2, :],
                    )
                t1 = rt.tile([128, 512], MM_DT, tag="rt")
                t2 = rt.tile([128, 512], MM_DT, tag="rt")
                nc.vector.tensor_tensor(out=t1[:], in0=raw_sb[:], in1=cos2[:, ssl], op=Alu.mult)
                nc.vector.tensor_tensor(out=t2[:], in0=rot_sb[:], in1=sin2[:, ssl], op=Alu.mult)
                nc.vector.tensor_add(out=dest[hp][sc][:], in0=t1[:], in1=t2[:])
            return emit

        def v_quantum(st):
            """V projection for one kk tile st (natural layout + ones col)."""
            def emit():
                v_ps = psum.tile([128, JC], F32, tag="mm")
                for hc in range(NHC):
                    nc.tensor.matmul(
                        out=v_ps[:],
                        lhsT=hsT_c[hc][:, st * 128:(st + 1) * 128],
                        rhs=wv_c[hc][:],
                        start=(hc == 0), stop=(hc == NHC - 1),
                    )
                nc.vector.tensor_copy(
                    out=v_sb[st // 4][:, st % 4, :, 0:HD],
                    in_=v_ps[:].rearrange("p (h d) -> p h d", h=HPC),
                )
            return emit

        den_tiles = {}

        def attn_block(j, work=()):
            work_q = list(work)
            qsl = slice(j * 512, (j + 1) * 512)
            for hp in range(4):
                ctx_ps = [psum.tile([HD + 1, 512], F32, tag="ctx", name=f"ctx_ps{_i}")
                          for _i in range(2)]
                nt = 4 * j + 4
                for t in range(nt):
                    # causal trim: diagonal tile d only needs columns >= 128*d
                    d = t - 4 * j
                    c0 = 128 * d if d > 0 else 0
                    sc_ps = psum.tile([128, 2, 512], F32, tag="sc")
                    kt, ko = t // 4, (t % 4) * 128
                    for hl in range(2):
                        pr = slice(64 * hl, 64 * hl + 64)
                        nc.tensor.matmul(
                            out=sc_ps[:, hl, c0:512],
                            lhsT=krope[hp][kt][pr, ko:ko + 128],
                            rhs=qrope[hp][j][pr, c0:512],
                            start=True, stop=True,
                        )
                    pt = ptp.tile([128, 2, 512], MM_DT, tag="pt")
                    # q and k carry a W8S factor each from the fp8 weight scale
                    nc.scalar.activation(out=pt[:, :, c0:512], in_=sc_ps[:, :, c0:512],
                                         func=AF.Exp, scale=float(SCALE) / (W8S * W8S))
                    if d >= 0:
                        # mask the 128-wide diagonal triangle: keep q >= kk.
                        # Block 0 masks on the DVE so its critical path never
                        # waits on the gpsimd FIFO behind the CC warmups.
                        for hl in range(2):
                            if j == 0:
                                nc.vector.tensor_tensor(
                                    out=pt[:, hl, c0:c0 + 128],
                                    in0=pt[:, hl, c0:c0 + 128],
                                    in1=trimask[:], op=Alu.mult,
                                )
                            else:
                                nc.gpsimd.affine_select(
                                    out=pt[:, hl, c0:c0 + 128], in_=pt[:, hl, c0:c0 + 128],
                                    pattern=[[1, 128]], compare_op=Alu.is_ge,
                                    fill=0.0, base=0,
                                    channel_multiplier=-1,
                                )
                    for hl in range(2):
                        nc.tensor.matmul(
                            out=ctx_ps[hl][:, c0:512],
                            lhsT=v_sb[t // 4][:, t % 4, 2 * hp + hl, :],
                            rhs=pt[:, hl, c0:512],
                            start=(t == 0), stop=(t == nt - 1),
                        )
                    # fill this tile's exp-wait bubble with independent work
                    # (projections for a later chunk, finalize of an earlier
                    # one) -- the PE queue is strict FIFO, so overlap only
                    # happens if the filler is emitted between attention tiles
                    if work_q:
                        work_q.pop(0)()
                den = misc.tile([128, 512], MM_DT, tag="srow", bufs=4,
                                name=f"den{j}_{hp}")
                for hl in range(2):
                    pr = slice(64 * hl, 64 * hl + 64)
                    nc.vector.tensor_copy(out=ctx_sb[hp][pr, qsl], in_=ctx_ps[hl][0:HD, :])
                    nc.vector.tensor_copy(out=den[64 * hl:64 * hl + 1, :],
                                          in_=ctx_ps[hl][HD:HD + 1, :])
                den_tiles.setdefault(j, []).append(den)
            # drain any leftover filler work
            while work_q:
                work_q.pop(0)()

        def norm_quantum(j, hp):
            # deferred so the bc matmul never head-of-line blocks the PE queue
            # behind the just-issued den copies
            qsl = slice(j * 512, (j + 1) * 512)

            def emit():
                den = den_tiles[j][hp]
                bc_ps = psum.tile([128, 512], F32, tag="mm", name=f"bcps{j}_{hp}")
                for hl in range(2):
                    nc.tensor.matmul(
                        out=bc_ps[64 * hl:64 * hl + 64, :],
                        lhsT=ones1[64 * hl:64 * hl + 1, 0:64],
                        rhs=den[64 * hl:64 * hl + 1, :],
                        start=True, stop=True,
                    )
                bc = misc.tile([128, 512], F32, tag="bc", bufs=5, name=f"bc{j}_{hp}")
                nc.vector.reciprocal_approx_fast(out=bc[:], in_=bc_ps[:])
                nc.vector.tensor_tensor(
                    out=ctx_sb[hp][:, qsl], in0=ctx_sb[hp][:, qsl], in1=bc[:], op=Alu.mult,
                )
            return emit

        def o_quantum(st, jc2):
            def emit():
                ssl2 = slice(st * 128, (st + 1) * 128)
                osl = slice(jc2 * 512, (jc2 + 1) * 512)
                o_ps = psum.tile([128, 512], F32, tag="mm")
                for kc in range(4):
                    nc.tensor.matmul(
                        out=o_ps[:],
                        lhsT=ctx_sb[kc][:, ssl2],
                        rhs=wo[:, kc, osl],
                        start=(kc == 0), stop=(kc == 3),
                    )
                o_sb = outp.tile([128, 512], MM_DT, tag="osb")
                nc.vector.tensor_copy(out=o_sb[:], in_=o_ps[:])
                nc.sync.dma_start(out=cc_in[ssl2, osl], in_=o_sb[:])
            return emit

        def fin_quanta(j, sts=None):
            q = [norm_quantum(j, hp) for hp in range(4)] if sts is None else []
            for st in (range(4 * j, 4 * j + 4) if sts is None else sts):
                for jc2 in range(2):
                    q.append(o_quantum(st, jc2))
            return q

        def rs(lo, hi, out_t):
            nc.gpsimd.collective_compute(
                "ReduceScatter", Alu.add,
                replica_groups=[[0, 1], [2, 3], [4, 5], [6, 7]],
                ins=[cc_in[lo:hi, :]], outs=[out_t[:]],
            )

        # warm up the collective path while the PE chews projections; the
        # cold-start cost (observed 15-160us, random per core) tends to hit
        # the first couple of ops, so burn several tiny ones
        for _ in range(3):
            nc.gpsimd.collective_compute(
                "ReduceScatter", Alu.add,
                replica_groups=[[0, 1], [2, 3], [4, 5], [6, 7]],
                ins=[cc_warm_in[:]], outs=[cc_warm_out[:]],
            )

        # Serial preamble: chunk-0 projections with the first V tiles
        # interleaved so attention block 0's ctx matmuls aren't left waiting
        # on V at the end of the preamble.
        for hp in range(4):
            proj_quantum(0, hp, "q", act_copy=True)()
            proj_quantum(0, hp, "k", act_copy=True)()
            v_quantum(hp)()

        # Pipelined emission: each attention block's exp-wait bubbles are
        # filled with the next chunk's projections and the previous chunk's
        # finalize, emitted tile-by-tile into the PE queue.
        attn_block(0, [proj_quantum(1, hp, w) for hp in range(4) for w in "qk"]
                      + [v_quantum(st) for st in range(4, 8)])
        attn_block(1, fin_quanta(0)
                      + [proj_quantum(2, hp, w) for hp in range(4) for w in "qk"]
                      + [v_quantum(st) for st in range(8, 12)])
        rs(0, 512, cc_out[0])
        attn_block(2, fin_quanta(1)
                      + [proj_quantum(3, hp, w) for hp in range(4) for w in "qk"]
                      + [v_quantum(st) for st in range(12, 16)])
        rs(512, 1024, cc_out[1])
        attn_block(3, fin_quanta(2))
        rs(1024, 1536, cc_out[2])
        for hp in range(4):
            norm_quantum(3, hp)()
        for qq, st in enumerate(range(12, 16)):
            for q in fin_quanta(3, sts=[st]):
                q()
            rs(1536 + 128 * qq, 1536 + 128 * (qq + 1), cc_out3[qq])
        for j in range(NJ - 1):
            nc.sync.dma_start(
                out=out_d[j * 256:(j + 1) * 256, :], in_=cc_out[j][:],
            )
        for qq in range(4):
            nc.sync.dma_start(out=out_d[768 + 64 * qq:768 + 64 * (qq + 1), :],
                              in_=cc_out3[qq][:])

    nc.finalize()
    return nc


def _rope_tables():
    inv_freq = (1.0 / (ROPE_BASE ** (np.arange(0, HD, 2, dtype=np.float32) / np.float32(HD)))).astype(np.float32)
    t = np.arange(S, dtype=np.float32)
    freqs = np.outer(t, inv_freq).astype(np.float32)          # [S, 32]
    emb = np.concatenate([freqs, freqs], axis=-1)             # [S, 64]
    return np.cos(emb).astype(np.float32), np.sin(emb).astype(np.float32)


def prepare_in_maps(hidden_states, Wq, Wk, Wv, Wo):
    cos, sin = _rope_tables()                                  # [S, 64]
    cos2 = np.ascontiguousarray(np.tile(cos.T, (2, 1)))        # [128, S]
    sin2 = np.ascontiguousarray(np.tile(sin.T, (2, 1)))
    # sign of the rotation (-x2 for d<32) folded into the sin table
    sin2[0:32] *= -1.0
    sin2[64:96] *= -1.0
    import ml_dtypes
    if MM_DT == F16:
        f16 = np.float16
    else:
        f16 = ml_dtypes.bfloat16
    f8 = ml_dtypes.float8_e4m3
    in_maps = []
    for c in range(NCORES):
        b, hg = c // 2, c % 2
        sl = slice(JC * hg, JC * (hg + 1))
        hsTb = np.ascontiguousarray(hidden_states[b].T)
        m = {
            "hsT": hsTb.astype(f16),
            "hsT8": hsTb.astype(f8),
            "wq8T": np.ascontiguousarray(Wq[sl].T * W8S).astype(f8),
            "wk8T": np.ascontiguousarray(Wk[sl].T * W8S).astype(f8),
            "wvT": np.ascontiguousarray(Wv[sl].T).astype(f16),
            "woT": np.ascontiguousarray(Wo[:, sl].T).astype(f16),
            "cosT2": cos2.astype(f16),
            "sinT2": sin2.astype(f16),
            "trimask": np.triu(np.ones((128, 128), dtype=np.float32)).astype(f16),
        }
        in_maps.append(m)
    return in_maps


def run(inputs, trace=False, tmpdir=None):
    global _PROGRAM
    if _PROGRAM is None:
        _PROGRAM = build()
    nc = _PROGRAM
    in_maps = prepare_in_maps(
        np.asarray(inputs["hidden_states"], dtype=np.float32),
        np.asarray(inputs["Wq"], dtype=np.float32),
        np.asarray(inputs["Wk"], dtype=np.float32),
        np.asarray(inputs["Wv"], dtype=np.float32),
        np.asarray(inputs["Wo"], dtype=np.float32),
    )
    res = run_bass_kernel_spmd(nc, in_maps, list(range(NCORES)), trace=trace, tmpdir=tmpdir)
    out = np.empty((B, S, HID), dtype=np.float32)
    for b in range(B):
        lo = np.asarray(res.results[2 * b]["out"], dtype=np.float32)
        hi = np.asarray(res.results[2 * b + 1]["out"], dtype=np.float32)
        for j in range(NJ - 1):
            out[b, 512 * j:512 * j + 256] = lo[256 * j:256 * (j + 1)]
            out[b, 512 * j + 256:512 * (j + 1)] = hi[256 * j:256 * (j + 1)]
        # chunk 3's ReduceScatter ran as four 128-row quarters
        for qq in range(4):
            base = 1536 + 128 * qq
            out[b, base:base + 64] = lo[768 + 64 * qq:768 + 64 * (qq + 1)]
            out[b, base + 64:base + 128] = hi[768 + 64 * qq:768 + 64 * (qq + 1)]
    return out, res


def kernel(**inputs):
    out, _ = run(inputs)
    return out


# revision 50
# speedup vs baseline: 1.1051x; 1.1051x over previous
"""Causal multi-head attention (B=4, S=2048, HID=1024, 16 heads x 64) with RoPE
on 8 TRN2 NeuronCores.

Sharding: core c -> batch b = c//2, head-group hg = c%2 (8 heads each).

Per core: Q/K projections are emitted in four per-q-chunk stages interleaved
with the attention blocks, so projection matmuls fill the PE while the ACT
engine grinds through the exp()s of the previous attention block. Scores are
computed transposed sT[kk, q] with both heads of a pair row-packed into one PE
pass, exp on ACT trimmed to the causal column range, causal masking via
affine_select on the 128-wide diagonal triangle only, V carries a ones column
so the softmax denominator lands in ctx psum row 64, denominator broadcast via
K=1 ones-matmuls on the PE (no DRAM bounce), o_proj and a pair ReduceScatter
(fp16) pipelined per 512-query chunk with the last chunk's RS split in half to
shorten the tail.

All matmuls run in fp16 (fp32 PSUM accumulation). Constants live in per-chunk
tiles so consumers only wait for the DMAs they actually need.
"""
import os as _os
import numpy as np
from contextlib import ExitStack

import concourse.bass as bass
import concourse.tile as tile
import concourse.mybir as mybir
from concourse import bacc
from concourse.alu_op_type import AluOpType
from concourse.bass_utils import run_bass_kernel_spmd
import concourse.bass_utils as _bu

LDW_OPT = _os.environ.get("KLDW", "0") == "1"
if LDW_OPT and not getattr(_bu, "_ldw_patched", False):
    _orig_run_command = _bu.run_command
    def _run_command_ldwopt(argv, **kw):
        argv = ["--enable-ldw-opt=true" if a == "--enable-ldw-opt=false" else a
                for a in argv]
        return _orig_run_command(argv, **kw)
    _bu.run_command = _run_command_ldwopt
    _bu._ldw_patched = True

F32 = mybir.dt.float32
F16 = mybir.dt.float16
BF16 = mybir.dt.bfloat16
F8 = mybir.dt.float8e4
MM_DT = BF16 if _os.environ.get("KMM", "f16") == "bf16" else F16
DR = mybir.MatmulPerfMode.DoubleRow
AF = mybir.ActivationFunctionType
Alu = AluOpType
W8S = 32.0       # fp8 Q/K weight pre-scale (keeps sigma~0.02 out of subnormals)

B, S, HID = 4, 2048, 1024
NH, HD = 16, 64
SCALE = 1.0 / np.sqrt(HD)
ROPE_BASE = 10000.0
NCORES = 8
HPC = 8          # heads per core
JC = 512         # head dims per core
NJ = 4           # q chunks of 512
NT = 16          # kk tiles of 128
NSC = 4          # s chunks of 512 for projections
NHC = 8          # hid chunks of 128 (contraction)

_PROGRAM = None


def build():
    nc = bacc.Bacc("TRN2", target_bir_lowering=False, debug=False)

    hsT_d = nc.declare_dram_parameter("hsT", [HID, S], MM_DT, isOutput=False)
    hsT8_d = nc.declare_dram_parameter("hsT8", [HID, S], F8, isOutput=False)
    wq8_d = nc.declare_dram_parameter("wq8T", [HID, JC], F8, isOutput=False)
    wk8_d = nc.declare_dram_parameter("wk8T", [HID, JC], F8, isOutput=False)
    wv_d = nc.declare_dram_parameter("wvT", [HID, JC], MM_DT, isOutput=False)
    wo_d = nc.declare_dram_parameter("woT", [JC, HID], MM_DT, isOutput=False)
    cos_d = nc.declare_dram_parameter("cosT2", [128, S], MM_DT, isOutput=False)
    sin_d = nc.declare_dram_parameter("sinT2", [128, S], MM_DT, isOutput=False)
    trimask_d = nc.declare_dram_parameter("trimask", [128, 128], MM_DT, isOutput=False)
    out_d = nc.declare_dram_parameter("out", [S // 2, HID], MM_DT, isOutput=True)

    cc_in = nc.dram_tensor("cc_in", [S, HID], MM_DT)
    # last q chunk's ReduceScatter is split in four to shorten the tail
    cc_out = [nc.dram_tensor(f"cc_out{j}", [S // 8, HID], MM_DT) for j in range(NJ - 1)]
    cc_out3 = [nc.dram_tensor(f"cc_out3{h}", [S // 32, HID], MM_DT) for h in range(4)]
    # tiny scratch collective fired at t~0: absorbs the CC-path init cost
    # (observed 15-100us on the first real collective) during the projections
    cc_warm_in = nc.dram_tensor("cc_warm_in", [2, 128], MM_DT)
    cc_warm_out = nc.dram_tensor("cc_warm_out", [1, 128], MM_DT)

    with ExitStack() as ctx:
        tc = ctx.enter_context(tile.TileContext(nc, num_cores=NCORES))
        consts = ctx.enter_context(tc.tile_pool(name="consts", bufs=1))
        rt = ctx.enter_context(tc.tile_pool(name="rt", bufs=6))
        ptp = ctx.enter_context(tc.tile_pool(name="ptp", bufs=6))
        misc = ctx.enter_context(tc.tile_pool(name="misc", bufs=2))
        outp = ctx.enter_context(tc.tile_pool(name="outp", bufs=5))
        psum = ctx.enter_context(tc.tile_pool(name="psum", bufs=2, space="PSUM"))

        # ---- constants in per-chunk tiles (fine-grained DMA deps) ----
        # Q/K projections run in fp8-DR (pair layout [128, 2, .]); V needs
        # fp16 accuracy so hsT is shipped in both precisions.
        hsT8_c = [consts.tile([128, 2, S], F8, tag=f"hsT8{c}", name=f"hsT8{c}")
                  for c in range(NHC // 2)]
        wq8_c = [consts.tile([128, 2, JC], F8, tag=f"wq8{c}", name=f"wq8{c}")
                 for c in range(NHC // 2)]
        wk8_c = [consts.tile([128, 2, JC], F8, tag=f"wk8{c}", name=f"wk8{c}")
                 for c in range(NHC // 2)]
        hsT_c = [consts.tile([128, S], MM_DT, tag=f"hsT{hc}", name=f"hsT{hc}")
                 for hc in range(NHC)]
        wv_c = [consts.tile([128, JC], MM_DT, tag=f"wv{hc}", name=f"wv{hc}")
                for hc in range(NHC)]
        # round-robin the issue queues so the boot DMA stream isn't paced by
        # a single engine's ~0.6us-per-descriptor issue rate
        _qs = [nc.sync, nc.gpsimd, nc.scalar]
        _qi = [0]
        def _dma(out, in_):
            _qs[_qi[0] % 3].dma_start(out=out, in_=in_)
            _qi[0] += 1
        for c in range(NHC // 2):
            psl = slice(c * 256, (c + 1) * 256)
            _dma(wq8_c[c][:], wq8_d[psl, :].rearrange("(o p) j -> p o j", p=128))
            _dma(hsT8_c[c][:], hsT8_d[psl, :].rearrange("(o p) j -> p o j", p=128))
            _dma(wk8_c[c][:], wk8_d[psl, :].rearrange("(o p) j -> p o j", p=128))
        for hc in range(NHC):
            csl = slice(hc * 128, (hc + 1) * 128)
            _dma(hsT_c[hc][:], hsT_d[csl, :])
            _dma(wv_c[hc][:], wv_d[csl, :])
        cos2 = consts.tile([128, S], MM_DT, tag="cos2")
        sin2 = consts.tile([128, S], MM_DT, tag="sin2")
        _dma(cos2[:], cos_d[:])
        _dma(sin2[:], sin_d[:])
        # upper triangle (keep q >= kk) for DVE-side masking of block 0,
        # whose gpsimd affine_selects would otherwise queue behind the
        # collective warmup triggers on the gpsimd FIFO
        trimask = consts.tile([128, 128], MM_DT, tag="trimask")
        _dma(trimask[:], trimask_d[:])
        wo = consts.tile([128, 4, HID], MM_DT, tag="wo")
        _dma(wo[:], wo_d[:].rearrange("(c p) j -> p c j", p=128))
        # all-ones stationary tile: K=1 matmuls replicate a denominator row
        # across 64 output partitions
        ones1 = consts.tile([128, 128], MM_DT, tag="ones1")
        nc.vector.memset(ones1[:], 1.0)

        # rope outputs in per-q-chunk tiles so a later projection stage's
        # writes never false-conflict with an earlier attention block's reads
        qrope = [[consts.tile([128, 512], MM_DT, tag=f"qr{i}_{sc}",
                              name=f"qrope{i}_{sc}") for sc in range(NSC)]
                 for i in range(4)]
        krope = [[consts.tile([128, 512], MM_DT, tag=f"kr{i}_{sc}",
                              name=f"krope{i}_{sc}") for sc in range(NSC)]
                 for i in range(4)]
        # V in per-st-group tiles (group g holds kk tiles 4g..4g+3)
        v_sb = [consts.tile([128, 4, HPC, HD + 1], MM_DT, tag=f"v_sb{g}",
                            name=f"v_sb{g}") for g in range(4)]
        for g in range(4):
            nc.vector.memset(v_sb[g][:, :, :, HD:HD + 1], 1.0)

        ctx_sb = [consts.tile([128, S], MM_DT, tag=f"ctx{i}", name=f"ctx_sb{i}")
                  for i in range(4)]

        def proj_quantum(sc, hp, which, act_copy=False):
            """One Q or K projection group + RoPE for (q-chunk sc, head pair hp)."""
            ssl = slice(sc * 512, (sc + 1) * 512)
            jcol = hp * 128
            wt = wq8_c if which == "q" else wk8_c
            dest = qrope if which == "q" else krope

            def emit():
                ps_raw = psum.tile([128, 512], F32, tag="mm")
                for c in range(NHC // 2):
                    nc.tensor.matmul(
                        out=ps_raw[:],
                        lhsT=wt[c][:, :, jcol:jcol + 128],
                        rhs=hsT8_c[c][:, :, ssl],
                        start=(c == 0), stop=(c == NHC // 2 - 1),
                        perf_mode=DR,
                    )
                raw_sb = misc.tile([128, 512], MM_DT, tag="qraw", bufs=4,
                                   name=f"raw_{which}{hp}_{sc}")
                # preamble drains via ACT (idle there); pumped stages via DVE
                # so the psum rotation never waits behind the exp grind
                if act_copy:
                    nc.scalar.copy(out=raw_sb[:], in_=ps_raw[:])
                else:
                    nc.vector.tensor_copy(out=raw_sb[:], in_=ps_raw[:])
                rot_sb = misc.tile([128, 512], MM_DT, tag="qrot", bufs=4,
                                   name=f"rot_{which}{hp}_{sc}")
                # all rot permutes ride the sync queue: gpsimd may be held for
                # tens of us by the collective warmup triggers, and the scalar
                # queue must stay clear for the exp grind
                alt_q = nc.scalar if act_copy else nc.sync
                for hl in range(2):
                    b0 = 64 * hl
                    # rot rows 0:32 <- raw rows 1,3,..,63 (odd)
                    nc.sync.dma_start(
                        out=rot_sb[b0:b0 + 32, :],
                        in_=raw_sb[b0 + 1:b0 + 64:2, :],
                    )
                    alt_q.dma_start(
                        out=rot_sb[b0 + 32:b0 + 64, :],
                        in_=raw_sb[b0:b0 + 63:2, :],
                    )
                t1 = rt.tile([128, 512], MM_DT, tag="rt")
                t2 = rt.tile([128, 512], MM_DT, tag="rt")
                nc.vector.tensor_tensor(out=t1[:], in0=raw_sb[:], in1=cos2[:, ssl], op=Alu.mult)
                nc.vector.tensor_tensor(out=t2[:], in0=rot_sb[:], in1=sin2[:, ssl], op=Alu.mult)
                nc.vector.tensor_add(out=dest[hp][sc][:], in0=t1[:], in1=t2[:])
            return emit

        def v_quantum(st):
            """V projection for one kk tile st (natural layout + ones col)."""
            def emit():
                v_ps = psum.tile([128, JC], F32, tag="mm")
                for hc in range(NHC):
                    nc.tensor.matmul(
                        out=v_ps[:],
                        lhsT=hsT_c[hc][:, st * 128:(st + 1) * 128],
                        rhs=wv_c[hc][:],
                        start=(hc == 0), stop=(hc == NHC - 1),
                    )
                nc.vector.tensor_copy(
                    out=v_sb[st // 4][:, st % 4, :, 0:HD],
                    in_=v_ps[:].rearrange("p (h d) -> p h d", h=HPC),
                )
            return emit

        den_tiles = {}

        def attn_block(j, work=()):
            work_q = list(work)
            qsl = slice(j * 512, (j + 1) * 512)
            for hp in range(4):
                ctx_ps = [psum.tile([HD + 1, 512], F32, tag="ctx", name=f"ctx_ps{_i}")
                          for _i in range(2)]
                nt = 4 * j + 4
                for t in range(nt):
                    # causal trim: diagonal tile d only needs columns >= 128*d
                    d = t - 4 * j
                    c0 = 128 * d if d > 0 else 0
                    sc_ps = psum.tile([128, 2, 512], F32, tag="sc")
                    kt, ko = t // 4, (t % 4) * 128
                    for hl in range(2):
                        pr = slice(64 * hl, 64 * hl + 64)
                        nc.tensor.matmul(
                            out=sc_ps[:, hl, c0:512],
                            lhsT=krope[hp][kt][pr, ko:ko + 128],
                            rhs=qrope[hp][j][pr, c0:512],
                            start=True, stop=True,
                        )
                    pt = ptp.tile([128, 2, 512], MM_DT, tag="pt")
                    # q and k carry a W8S factor each from the fp8 weight scale
                    nc.scalar.activation(out=pt[:, :, c0:512], in_=sc_ps[:, :, c0:512],
                                         func=AF.Exp, scale=float(SCALE) / (W8S * W8S))
                    if d >= 0:
                        # mask the 128-wide diagonal triangle: keep q >= kk.
                        # Blocks 0-1 mask on the DVE so their critical path
                        # never waits on the gpsimd FIFO behind the CC warmups.
                        for hl in range(2):
                            if j <= 1:
                                nc.vector.tensor_tensor(
                                    out=pt[:, hl, c0:c0 + 128],
                                    in0=pt[:, hl, c0:c0 + 128],
                                    in1=trimask[:], op=Alu.mult,
                                )
                            else:
                                nc.gpsimd.affine_select(
                                    out=pt[:, hl, c0:c0 + 128], in_=pt[:, hl, c0:c0 + 128],
                                    pattern=[[1, 128]], compare_op=Alu.is_ge,
                                    fill=0.0, base=0,
                                    channel_multiplier=-1,
                                )
                    for hl in range(2):
                        nc.tensor.matmul(
                            out=ctx_ps[hl][:, c0:512],
                            lhsT=v_sb[t // 4][:, t % 4, 2 * hp + hl, :],
                            rhs=pt[:, hl, c0:512],
                            start=(t == 0), stop=(t == nt - 1),
                        )
                    # fill this tile's exp-wait bubble with independent work
                    # (projections for a later chunk, finalize of an earlier
                    # one) -- the PE queue is strict FIFO, so overlap only
                    # happens if the filler is emitted between attention tiles
                    if work_q:
                        work_q.pop(0)()
                den = misc.tile([128, 512], MM_DT, tag="srow", bufs=4,
                                name=f"den{j}_{hp}")
                for hl in range(2):
                    pr = slice(64 * hl, 64 * hl + 64)
                    nc.vector.tensor_copy(out=ctx_sb[hp][pr, qsl], in_=ctx_ps[hl][0:HD, :])
                    nc.vector.tensor_copy(out=den[64 * hl:64 * hl + 1, :],
                                          in_=ctx_ps[hl][HD:HD + 1, :])
                den_tiles.setdefault(j, []).append(den)
            # drain any leftover filler work
            while work_q:
                work_q.pop(0)()

        def norm_quantum(j, hp):
            # deferred so the bc matmul never head-of-line blocks the PE queue
            # behind the just-issued den copies
            qsl = slice(j * 512, (j + 1) * 512)

            def emit():
                den = den_tiles[j][hp]
                bc_ps = psum.tile([128, 512], F32, tag="mm", name=f"bcps{j}_{hp}")
                for hl in range(2):
                    nc.tensor.matmul(
                        out=bc_ps[64 * hl:64 * hl + 64, :],
                        lhsT=ones1[64 * hl:64 * hl + 1, 0:64],
                        rhs=den[64 * hl:64 * hl + 1, :],
                        start=True, stop=True,
                    )
                bc = misc.tile([128, 512], F32, tag="bc", bufs=5, name=f"bc{j}_{hp}")
                nc.vector.reciprocal_approx_fast(out=bc[:], in_=bc_ps[:])
                nc.vector.tensor_tensor(
                    out=ctx_sb[hp][:, qsl], in0=ctx_sb[hp][:, qsl], in1=bc[:], op=Alu.mult,
                )
            return emit

        def o_quantum(st, jc2):
            def emit():
                ssl2 = slice(st * 128, (st + 1) * 128)
                osl = slice(jc2 * 512, (jc2 + 1) * 512)
                o_ps = psum.tile([128, 512], F32, tag="mm")
                for kc in range(4):
                    nc.tensor.matmul(
                        out=o_ps[:],
                        lhsT=ctx_sb[kc][:, ssl2],
                        rhs=wo[:, kc, osl],
                        start=(kc == 0), stop=(kc == 3),
                    )
                o_sb = outp.tile([128, 512], MM_DT, tag="osb")
                nc.vector.tensor_copy(out=o_sb[:], in_=o_ps[:])
                nc.sync.dma_start(out=cc_in[ssl2, osl], in_=o_sb[:])
            return emit

        def fin_quanta(j, sts=None):
            q = [norm_quantum(j, hp) for hp in range(4)] if sts is None else []
            for st in (range(4 * j, 4 * j + 4) if sts is None else sts):
                for jc2 in range(2):
                    q.append(o_quantum(st, jc2))
            return q

        def rs(lo, hi, out_t):
            nc.gpsimd.collective_compute(
                "ReduceScatter", Alu.add,
                replica_groups=[[0, 1], [2, 3], [4, 5], [6, 7]],
                ins=[cc_in[lo:hi, :]], outs=[out_t[:]],
            )

        # warm up the collective path while the PE chews projections; the
        # cold-start cost (observed 15-160us, random per core) tends to hit
        # the first couple of ops, so burn several tiny ones
        for _ in range(3):
            nc.gpsimd.collective_compute(
                "ReduceScatter", Alu.add,
                replica_groups=[[0, 1], [2, 3], [4, 5], [6, 7]],
                ins=[cc_warm_in[:]], outs=[cc_warm_out[:]],
            )

        # Serial preamble: chunk-0 projections with the first V tiles
        # interleaved so attention block 0's ctx matmuls aren't left waiting
        # on V at the end of the preamble.
        for hp in range(4):
            proj_quantum(0, hp, "q", act_copy=True)()
            proj_quantum(0, hp, "k", act_copy=True)()
            v_quantum(hp)()

        # Pipelined emission: each attention block's exp-wait bubbles are
        # filled with the next chunk's projections and the previous chunk's
        # finalize, emitted tile-by-tile into the PE queue.
        attn_block(0, [proj_quantum(1, hp, w) for hp in range(4) for w in "qk"]
                      + [v_quantum(st) for st in range(4, 8)])
        attn_block(1, fin_quanta(0)
                      + [proj_quantum(2, hp, w) for hp in range(4) for w in "qk"]
                      + [v_quantum(st) for st in range(8, 12)])
        rs(0, 512, cc_out[0])
        attn_block(2, fin_quanta(1)
                      + [proj_quantum(3, hp, w) for hp in range(4) for w in "qk"]
                      + [v_quantum(st) for st in range(12, 16)])
        rs(512, 1024, cc_out[1])
        attn_block(3, fin_quanta(2))
        rs(1024, 1536, cc_out[2])
        for hp in range(4):
            norm_quantum(3, hp)()
        for qq, st in enumerate(range(12, 16)):
            for q in fin_quanta(3, sts=[st]):
                q()
            rs(1536 + 128 * qq, 1536 + 128 * (qq + 1), cc_out3[qq])
        for j in range(NJ - 1):
            nc.sync.dma_start(
                out=out_d[j * 256:(j + 1) * 256, :], in_=cc_out[j][:],
            )
        for qq in range(4):
            nc.sync.dma_start(out=out_d[768 + 64 * qq:768 + 64 * (qq + 1), :],
                              in_=cc_out3[qq][:])

    nc.finalize()
    return nc


def _rope_tables():
    inv_freq = (1.0 / (ROPE_BASE ** (np.arange(0, HD, 2, dtype=np.float32) / np.float32(HD)))).astype(np.float32)
    t = np.arange(S, dtype=np.float32)
    freqs = np.outer(t, inv_freq).astype(np.float32)          # [S, 32]
    emb = np.concatenate([freqs, freqs], axis=-1)             # [S, 64]
    return np.cos(emb).astype(np.float32), np.sin(emb).astype(np.float32)


def prepare_in_maps(hidden_states, Wq, Wk, Wv, Wo):
    cos, sin = _rope_tables()                                  # [S, 64]
    cos2 = np.ascontiguousarray(np.tile(cos.T, (2, 1)))        # [128, S]
    sin2 = np.ascontiguousarray(np.tile(sin.T, (2, 1)))
    # sign of the rotation (-x2 for d<32) folded into the sin table
    sin2[0:32] *= -1.0
    sin2[64:96] *= -1.0
    import ml_dtypes
    if MM_DT == F16:
        f16 = np.float16
    else:
        f16 = ml_dtypes.bfloat16
    f8 = ml_dtypes.float8_e4m3
    in_maps = []
    for c in range(NCORES):
        b, hg = c // 2, c % 2
        sl = slice(JC * hg, JC * (hg + 1))
        hsTb = np.ascontiguousarray(hidden_states[b].T)
        m = {
            "hsT": hsTb.astype(f16),
            "hsT8": hsTb.astype(f8),
            "wq8T": np.ascontiguousarray(Wq[sl].T * W8S).astype(f8),
            "wk8T": np.ascontiguousarray(Wk[sl].T * W8S).astype(f8),
            "wvT": np.ascontiguousarray(Wv[sl].T).astype(f16),
            "woT": np.ascontiguousarray(Wo[:, sl].T).astype(f16),
            "cosT2": cos2.astype(f16),
            "sinT2": sin2.astype(f16),
            "trimask": np.triu(np.ones((128, 128), dtype=np.float32)).astype(f16),
        }
        in_maps.append(m)
    return in_maps


def run(inputs, trace=False, tmpdir=None):
    global _PROGRAM
    if _PROGRAM is None:
        _PROGRAM = build()
    nc = _PROGRAM
    in_maps = prepare_in_maps(
        np.asarray(inputs["hidden_states"], dtype=np.float32),
        np.asarray(inputs["Wq"], dtype=np.float32),
        np.asarray(inputs["Wk"], dtype=np.float32),
        np.asarray(inputs["Wv"], dtype=np.float32),
        np.asarray(inputs["Wo"], dtype=np.float32),
    )
    res = run_bass_kernel_spmd(nc, in_maps, list(range(NCORES)), trace=trace, tmpdir=tmpdir)
    out = np.empty((B, S, HID), dtype=np.float32)
    for b in range(B):
        lo = np.asarray(res.results[2 * b]["out"], dtype=np.float32)
        hi = np.asarray(res.results[2 * b + 1]["out"], dtype=np.float32)
        for j in range(NJ - 1):
            out[b, 512 * j:512 * j + 256] = lo[256 * j:256 * (j + 1)]
            out[b, 512 * j + 256:512 * (j + 1)] = hi[256 * j:256 * (j + 1)]
        # chunk 3's ReduceScatter ran as four 128-row quarters
        for qq in range(4):
            base = 1536 + 128 * qq
            out[b, base:base + 64] = lo[768 + 64 * qq:768 + 64 * (qq + 1)]
            out[b, base + 64:base + 128] = hi[768 + 64 * qq:768 + 64 * (qq + 1)]
    return out, res


def kernel(**inputs):
    out, _ = run(inputs)
    return out
